# revision 7
# baseline (speedup 1.0000x reference)
"""Trainium2 Bass kernel for nn_BiLSTMNet (2-layer BiLSTM + pair-gather MLP).

Strategy: fully data-parallel across 8 cores (16 sentences each), both LSTM
directions fused per core via block-diagonal matmuls.  Input projections are
computed just-in-time into PSUM (128-slot chunks); the recurrent matmul
accumulates on top (start=False), so gate pre-activations never touch DVE.
h^T is produced by DMA-transpose (bf16) and stored to DRAM in both processing
and reverse order so layer-1 / MLP consumers always read ascending columns.
MLP is decomposed as U0 = h1 @ w1[:, :2H].T, U1 = h1 @ w1[:, 2H:].T computed
for all (t, b), then the conf-pair gather is a row gather + add + tanh.
"""
import sys
sys.path.insert(0, "/opt/trn_rl_repo")
import numpy as np
import ml_dtypes

import concourse.bass as bass
import concourse.tile as tile
from concourse import mybir, bacc
from concourse.bass_utils import run_bass_kernel_spmd

BF16 = mybir.dt.bfloat16
F32 = mybir.dt.float32
I32 = mybir.dt.int32
AF = mybir.ActivationFunctionType
ALU = mybir.AluOpType

V, E, H, B, C = 32000, 200, 200, 128, 256
T_FULL = 512
BL = 16            # sentences per core
NCORE = 8
EP = 256           # padded E (dma-transpose wants 128-col blocks)
HP = 256           # padded H
G4 = 800           # 4*H gate width
CHT = 4            # timesteps per xg chunk (chunk = CHT*2*BL = 128 slots)


def build(T, n_cores, NPT):
    NCH = T // CHT
    NSLOT = T * BL                # per-direction (t,b) slots
    NUC = NSLOT // 128            # U-phase chunks

    nc = bacc.Bacc("TRN2", target_bir_lowering=False, debug=False,
                   enable_asserts=True, num_devices=n_cores)

    def din(name, shape, dt):
        return nc.dram_tensor(name, shape, dt, kind="ExternalInput").ap()

    def dout(name, shape, dt):
        return nc.dram_tensor(name, shape, dt, kind="ExternalOutput").ap()

    emb = din("emb", [V, E], BF16)
    W0s = din("W0s", [2 * EP, G4], BF16)      # xg0 stream (block-diag K rows)
    Whh0s = din("Whh0s", [2 * HP, G4], BF16)  # L0 recurrent stream
    W1sf = din("W1sf", [2 * H + 1, G4], BF16)  # xg1 stream, fwd block
    W1sb = din("W1sb", [2 * H + 1, G4], BF16)  # xg1 stream, bwd block
    Whh1s = din("Whh1s", [2 * HP, G4], BF16)
    WU = din("WU", [2 * H + 1, G4], BF16)      # U stream [w1a.T | w1b.T] + bw1 row
    W2s = din("W2s", [4 * 128, 4], BF16)       # w2.T padded to 512 rows + bw2 at 511
    tokf = din("tokf", [CHT * BL, NCH], I32)   # [slot, chunk]
    tokb = din("tokb", [CHT * BL, NCH], I32)
    uidx0 = din("uidx0", [128, NPT], I32)
    uidx1 = din("uidx1", [128, NPT], I32)
    umask0 = din("umask0", [128, NPT], F32)
    umask1 = din("umask1", [128, NPT], F32)
    bw1m = din("bw1m", [128, 2 * H], F32)

    OUT = dout("OUT", [NPT * 128, 4], F32)

    # internal DRAM
    # h0T rows: [0:200] f-proc | [200:400] b-rev | [400] ones |
    #           [401:601] f-rev | [601:801] b-proc | [801] ones
    h0T = nc.dram_tensor("h0T", [802, NSLOT], BF16).ap()
    # h1T rows: [0:200] f-proc | [200:400] b-rev | [400] ones
    h1T = nc.dram_tensor("h1T", [401, NSLOT], BF16).ap()
    U0 = nc.dram_tensor("U0", [NSLOT, 2 * H], F32).ap()
    U1 = nc.dram_tensor("U1", [NSLOT, 2 * H], F32).ap()

    with tile.TileContext(nc) as tc:
        with tc.tile_pool(name="const", bufs=1) as cp, \
             tc.tile_pool(name="state", bufs=1) as sp:

            # ---- load weight streams into SBUF K-chunk tiles
            def load_stream(src, nrows, ncols):
                tiles = []
                r = 0
                while r < nrows:
                    h_ = min(128, nrows - r)
                    t_ = cp.tile([h_, ncols], BF16, tag=f"st{src.name}{r}", name=f"st{src.name}{r}")
                    nc.sync.dma_start(out=t_[:], in_=src[r:r + h_, :])
                    tiles.append(t_)
                    r += h_
                return tiles

            W0t = load_stream(W0s.tensor.ap(), 2 * EP, G4)      # 4 x [128, 800]
            Whh0t = load_stream(Whh0s.tensor.ap(), 2 * HP, G4)  # 4
            W1ft = load_stream(W1sf.tensor.ap(), 2 * H + 1, G4)  # [128,128,128,17]
            W1bt = load_stream(W1sb.tensor.ap(), 2 * H + 1, G4)
            Whh1t = load_stream(Whh1s.tensor.ap(), 2 * HP, G4)
            WUt = load_stream(WU.tensor.ap(), 2 * H + 1, G4)
            W2t = load_stream(W2s.tensor.ap(), 4 * 128, 4)       # 4 x [128, 4]

            # token index tiles (slot-major: [64, NCH])
            tokf_t = cp.tile([CHT * BL, NCH], I32)
            tokb_t = cp.tile([CHT * BL, NCH], I32)
            nc.sync.dma_start(out=tokf_t[:], in_=tokf[:])
            nc.sync.dma_start(out=tokb_t[:], in_=tokb[:])

            # ones rows in h0T/h1T (bias rows consumed via lhsT chunk DMAs)
            ones_row = cp.tile([1, NSLOT], BF16)
            nc.vector.memset(ones_row[:], 1.0)
            nc.sync.dma_start(out=h0T[400:401, :], in_=ones_row[:])
            nc.sync.dma_start(out=h0T[801:802, :], in_=ones_row[:])
            nc.sync.dma_start(out=h1T[400:401, :], in_=ones_row[:])

            # ---- persistent state tiles
            # x gather tiles (per chunk parity): cols 200:255 zero, col 255 one
            xf = [sp.tile([CHT * BL, EP], BF16, tag=f"xf{i}", name=f"xf{i}") for i in range(2)]
            xb = [sp.tile([CHT * BL, EP], BF16, tag=f"xb{i}", name=f"xb{i}") for i in range(2)]
            for t_ in xf + xb:
                nc.vector.memset(t_[:], 0.0)
                nc.vector.memset(t_[:, EP - 1:EP], 1.0)
            # xg lhsT tiles (block-diag): C0..C3 per parity
            Ct = [[sp.tile([128, 128], BF16, tag=f"C{i}{j}", name=f"C{i}{j}") for i in range(4)]
                  for j in range(2)]
            # rec lhsT tiles A0..A3 per step parity
            At = [[sp.tile([128, 2 * BL], BF16, tag=f"A{i}{j}", name=f"A{i}{j}") for i in range(4)]
                  for j in range(2)]
            # L1 xg lhsT tiles D0..D7 per parity (last of each block is 17 rows)
            Dt = [[sp.tile([17 if i in (3, 7) else 128, 128], BF16, tag=f"D{i}{j}", name=f"D{i}{j}")
                   for i in range(8)] for j in range(2)]
            for j in range(2):
                for t_ in Ct[j] + At[j] + Dt[j]:
                    nc.vector.memset(t_[:], 0.0)
            # LSTM state: S = [c | tg] fp32; h per parity
            S = sp.tile([2 * BL, 2 * H], F32)
            ht = [sp.tile([2 * BL, HP], BF16, tag=f"h{i}", name=f"h{i}") for i in range(2)]
            for t_ in ht:
                nc.vector.memset(t_[:], 0.0)
            # hT chunk buffers (proc + reversed), per chunk parity
            hTb1 = [sp.tile([128, CHT * 2 * BL], BF16, tag=f"hTb1{i}", name=f"hTb1{i}") for i in range(2)]
            hTb2 = [sp.tile([128, CHT * 2 * BL], BF16, tag=f"hTb2{i}", name=f"hTb2{i}") for i in range(2)]
            hTr1 = [sp.tile([128, CHT * 2 * BL], BF16, tag=f"hTr1{i}", name=f"hTr1{i}") for i in range(2)]
            hTr2 = [sp.tile([128, CHT * 2 * BL], BF16, tag=f"hTr2{i}", name=f"hTr2{i}") for i in range(2)]

            NB = 2 * BL  # 32 rows per step (f+b)

            with tc.tile_pool(name="work", bufs=2) as wp, \
                 tc.tile_pool(name="xps", bufs=2, space="PSUM") as xps:

                xg_tiles = {}

                def emit_xg0_chunk(k):
                    par = k % 2
                    gxf = xf[par]
                    gxb = xb[par]
                    nc.gpsimd.indirect_dma_start(
                        out=gxf[:, 0:E], out_offset=None, in_=emb[:],
                        in_offset=bass.IndirectOffsetOnAxis(ap=tokf_t[:, k:k + 1], axis=0))
                    nc.gpsimd.indirect_dma_start(
                        out=gxb[:, 0:E], out_offset=None, in_=emb[:],
                        in_offset=bass.IndirectOffsetOnAxis(ap=tokb_t[:, k:k + 1], axis=0))
                    # transpose x -> staging, then strided copy into C tiles
                    for i, (src, coff) in enumerate([(gxf, 0), (gxb, BL)]):
                        for half in range(2):
                            stg = wp.tile([128, CHT * BL], BF16, tag="xstg", name="xstg")
                            nc.sync.dma_start_transpose(
                                out=stg[:], in_=src[:, half * 128:half * 128 + 128])
                            ctile = Ct[par][2 * i + half]
                            dst = ctile[:].rearrange("p (a b) -> p a b", b=NB)[:, :, coff:coff + BL]
                            s3 = stg[:].rearrange("p (a b) -> p a b", b=BL)
                            nc.vector.tensor_copy(dst, s3)
                    xg = xps.tile([128, G4], F32, space="PSUM", tag="xg", name="xg")
                    xg_tiles[("L0", k)] = xg
                    for kc in range(4):
                        for (n0, n1) in ((0, 512), (512, G4)):
                            nc.tensor.matmul(xg[:, n0:n1], Ct[par][kc][:],
                                             W0t[kc][:, n0:n1],
                                             start=(kc == 0), stop=(kc == 3))

                def emit_xg1_chunk(k):
                    par = k % 2
                    c0 = k * CHT * BL
                    cw = CHT * BL
                    rowsets = [(0, 128), (128, 256), (256, 384), (384, 401),
                               (401, 529), (529, 657), (657, 785), (785, 802)]
                    for i, (r0, r1) in enumerate(rowsets):
                        dtile = Dt[par][i]
                        coff = 0 if i < 4 else BL
                        dst = dtile[:].rearrange("p (a b) -> p a b", b=NB)[:, :, coff:coff + BL]
                        src = h0T[r0:r1, c0:c0 + cw].rearrange("p (a b) -> p a b", b=BL)
                        nc.sync.dma_start(out=dst, in_=src)
                    xg = xps.tile([128, G4], F32, space="PSUM", tag="xg", name="xg")
                    xg_tiles[("L1", k)] = xg
                    streams = [W1ft[0], W1ft[1], W1ft[2], W1ft[3],
                               W1bt[0], W1bt[1], W1bt[2], W1bt[3]]
                    for kc in range(8):
                        for (n0, n1) in ((0, 512), (512, G4)):
                            nc.tensor.matmul(xg[:, n0:n1], Dt[par][kc][:],
                                             streams[kc][:, n0:n1],
                                             start=(kc == 0), stop=(kc == 7))

                def emit_step(layer, p, T_, Whht, store_all):
                    par = p % 2
                    k = p // CHT
                    chp = k % 2
                    r = (p % CHT) * NB
                    xg = xg_tiles[(layer, k)]
                    # recurrent matmul accumulating onto the xg psum slice
                    for kc in range(4):
                        for (n0, n1) in ((0, 512), (512, G4)):
                            nc.tensor.matmul(xg[r:r + NB, n0:n1],
                                             At[(p + 1) % 2][kc][:],
                                             Whht[kc][:, n0:n1],
                                             start=False, stop=(kc == 3),
                                             skip_group_check=True,
                                             tile_position=(0, r))
                    # gate nonlinearities (gate order f,i,o,g)
                    sigs = wp.tile([NB, 600], F32, tag="sigs", name="sigs")
                    nc.scalar.activation(sigs[:], xg[r:r + NB, 0:600], AF.Sigmoid)
                    nc.scalar.activation(S[:, H:2 * H], xg[r:r + NB, 600:800], AF.Tanh)
                    prod = wp.tile([NB, 2 * H], F32, tag="prod", name="prod")
                    nc.vector.tensor_mul(prod[:], sigs[:, 0:2 * H], S[:, 0:2 * H])
                    nc.vector.tensor_add(S[:, 0:H], prod[:, 0:H], prod[:, H:2 * H])
                    tct = wp.tile([NB, H], F32, tag="tct", name="tct")
                    nc.scalar.activation(tct[:], S[:, 0:H], AF.Tanh)
                    hcur = ht[par]
                    nc.vector.tensor_mul(hcur[:, 0:H], sigs[:, 400:600], tct[:])
                    # transpose h -> hT buffers
                    t_rel = p % CHT
                    b1 = hTb1[chp]
                    b2 = hTb2[chp]
                    nc.sync.dma_start_transpose(
                        out=b1[:, t_rel * NB:(t_rel + 1) * NB], in_=hcur[:, 0:128])
                    nc.sync.dma_start_transpose(
                        out=b2[:, t_rel * NB:(t_rel + 1) * NB], in_=hcur[:, 128:256])
                    # reversed-order copies
                    rr = (CHT - 1 - t_rel) * NB
                    nc.vector.tensor_copy(hTr1[chp][:, rr:rr + NB],
                                          b1[:, t_rel * NB:(t_rel + 1) * NB])
                    nc.vector.tensor_copy(hTr2[chp][:, rr:rr + NB],
                                          b2[:, t_rel * NB:(t_rel + 1) * NB])
                    # next-step lhsT tiles
                    nA = At[par]
                    nc.vector.tensor_copy(nA[0][:, 0:BL],
                                          b1[:, t_rel * NB:t_rel * NB + BL])
                    nc.vector.tensor_copy(nA[1][:, 0:BL],
                                          b2[:, t_rel * NB:t_rel * NB + BL])
                    nc.vector.tensor_copy(nA[2][:, BL:NB],
                                          b1[:, t_rel * NB + BL:(t_rel + 1) * NB])
                    nc.vector.tensor_copy(nA[3][:, BL:NB],
                                          b2[:, t_rel * NB + BL:(t_rel + 1) * NB])
                    # end-of-chunk DRAM stores
                    if t_rel == CHT - 1:
                        hT = h0T if layer == "L0" else h1T
                        c0 = k * CHT * BL
                        cw = CHT * BL
                        rc0 = (T_ - CHT - k * CHT) * BL
                        def half(tl, lo):
                            return tl[:].rearrange("p (a b) -> p a b", b=NB)[:, :, lo:lo + BL]
                        # f-proc rows 0:200
                        nc.sync.dma_start(out=hT[0:128, c0:c0 + cw].rearrange("p (a b) -> p a b", b=BL),
                                          in_=half(b1, 0))
                        nc.sync.dma_start(out=hT[128:200, c0:c0 + cw].rearrange("p (a b) -> p a b", b=BL),
                                          in_=half(b2, 0)[0:72])
                        # b-rev rows 200:400 (reversed buffer, b half)
                        nc.sync.dma_start(out=hT[200:328, rc0:rc0 + cw].rearrange("p (a b) -> p a b", b=BL),
                                          in_=half(hTr1[chp], BL))
                        nc.sync.dma_start(out=hT[328:400, rc0:rc0 + cw].rearrange("p (a b) -> p a b", b=BL),
                                          in_=half(hTr2[chp], BL)[0:72])
                        if store_all:
                            # f-rev rows 401:601
                            nc.sync.dma_start(out=hT[401:529, rc0:rc0 + cw].rearrange("p (a b) -> p a b", b=BL),
                                              in_=half(hTr1[chp], 0))
                            nc.sync.dma_start(out=hT[529:601, rc0:rc0 + cw].rearrange("p (a b) -> p a b", b=BL),
                                              in_=half(hTr2[chp], 0)[0:72])
                            # b-proc rows 601:801
                            nc.sync.dma_start(out=hT[601:729, c0:c0 + cw].rearrange("p (a b) -> p a b", b=BL),
                                              in_=half(b1, BL))
                            nc.sync.dma_start(out=hT[729:801, c0:c0 + cw].rearrange("p (a b) -> p a b", b=BL),
                                              in_=half(b2, BL)[0:72])

                def reset_states():
                    nc.vector.memset(S[:], 0.0)
                    for j in range(2):
                        for t_ in At[j]:
                            nc.vector.memset(t_[:], 0.0)

                # ================= layer 0 =================
                reset_states()
                emit_xg0_chunk(0)
                for k in range(NCH):
                    if k + 1 < NCH:
                        emit_xg0_chunk(k + 1)
                    for tr in range(CHT):
                        emit_step("L0", k * CHT + tr, T, Whh0t, True)

                # ================= layer 1 =================
                reset_states()
                emit_xg1_chunk(0)
                for k in range(NCH):
                    if k + 1 < NCH:
                        emit_xg1_chunk(k + 1)
                    for tr in range(CHT):
                        emit_step("L1", k * CHT + tr, T, Whh1t, False)

            # ================= U phase =================
            with tc.tile_pool(name="uw", bufs=2) as uw, \
                 tc.tile_pool(name="ups", bufs=2, space="PSUM") as ups:
                rowsets = [(0, 128), (128, 256), (256, 384), (384, 401)]
                for k in range(NUC):
                    c0 = k * 128
                    et = []
                    for (r0, r1) in rowsets:
                        t_ = uw.tile([r1 - r0, 128], BF16, tag=f"E{r0}", name=f"E{r0}")
                        nc.sync.dma_start(out=t_[:], in_=h1T[r0:r1, c0:c0 + 128])
                        et.append(t_)
                    psu = ups.tile([128, G4], F32, space="PSUM", tag="psu", name="psu")
                    for kc in range(4):
                        for (n0, n1) in ((0, 512), (512, G4)):
                            nc.tensor.matmul(psu[:, n0:n1], et[kc][:],
                                             WUt[kc][:, n0:n1],
                                             start=(kc == 0), stop=(kc == 3))
                    uo = uw.tile([128, G4], F32, tag="uo", name="uo")
                    nc.vector.tensor_copy(uo[:], psu[:])
                    nc.sync.dma_start(out=U0[c0:c0 + 128, :], in_=uo[:, 0:2 * H])
                    nc.sync.dma_start(out=U1[c0:c0 + 128, :], in_=uo[:, 2 * H:G4])

            # ================= final gather + MLP =================
            with tc.tile_pool(name="fw", bufs=2) as fw, \
                 tc.tile_pool(name="fc", bufs=1) as fc, \
                 tc.tile_pool(name="fps", bufs=2, space="PSUM") as fps:
                ui0 = fc.tile([128, NPT], I32)
                ui1 = fc.tile([128, NPT], I32)
                um0 = fc.tile([128, NPT], F32)
                um1 = fc.tile([128, NPT], F32)
                nc.sync.dma_start(out=ui0[:], in_=uidx0[:])
                nc.sync.dma_start(out=ui1[:], in_=uidx1[:])
                nc.sync.dma_start(out=um0[:], in_=umask0[:])
                nc.sync.dma_start(out=um1[:], in_=umask1[:])
                bwt = fc.tile([128, 2 * H], F32, name="bwt")
                nc.sync.dma_start(out=bwt[:], in_=bw1m[:])
                hm = [fc.tile([128, 512], BF16, tag=f"hm{i}", name=f"hm{i}") for i in range(2)]
                for t_ in hm:
                    nc.vector.memset(t_[:], 0.0)
                    nc.vector.memset(t_[:, 511:512], 1.0)
                for j in range(NPT):
                    par = j % 2
                    g0 = fw.tile([128, 2 * H], F32, tag="g0", name="g0")
                    g1 = fw.tile([128, 2 * H], F32, tag="g1", name="g1")
                    nc.gpsimd.indirect_dma_start(
                        out=g0[:], out_offset=None, in_=U0[:],
                        in_offset=bass.IndirectOffsetOnAxis(ap=ui0[:, j:j + 1], axis=0))
                    nc.gpsimd.indirect_dma_start(
                        out=g1[:], out_offset=None, in_=U1[:],
                        in_offset=bass.IndirectOffsetOnAxis(ap=ui1[:, j:j + 1], axis=0))
                    g1m = fw.tile([128, 2 * H], F32, tag="g1m", name="g1m")
                    nc.vector.scalar_tensor_tensor(g1m[:], g1[:], um1[:, j:j + 1],
                                                   bwt[:], ALU.mult, ALU.add)
                    ssum = fw.tile([128, 2 * H], F32, tag="ssum", name="ssum")
                    nc.vector.scalar_tensor_tensor(ssum[:], g0[:], um0[:, j:j + 1],
                                                   g1m[:], ALU.mult, ALU.add)
                    nc.scalar.activation(hm[par][:, 0:2 * H], ssum[:], AF.Tanh)
                    hmT = []
                    for i in range(4):
                        t_ = fw.tile([128, 128], BF16, tag=f"hmT{i}", name=f"hmT{i}")
                        nc.sync.dma_start_transpose(
                            out=t_[:], in_=hm[par][:, i * 128:(i + 1) * 128])
                        hmT.append(t_)
                    psl = fps.tile([128, 4], F32, space="PSUM", tag="psl", name="psl")
                    for i in range(4):
                        nc.tensor.matmul(psl[:], hmT[i][:], W2t[i][:],
                                         start=(i == 0), stop=(i == 3))
                    ex = fw.tile([128, 4], F32, tag="ex", name="ex")
                    nc.scalar.activation(ex[:], psl[:], AF.Exp)
                    sm = fw.tile([128, 1], F32, tag="sm", name="sm")
                    nc.vector.reduce_sum(sm[:], ex[:], axis=mybir.AxisListType.X)
                    rc = fw.tile([128, 1], F32, tag="rc", name="rc")
                    nc.vector.reciprocal(rc[:], sm[:])
                    ot = fw.tile([128, 4], F32, tag="ot", name="ot")
                    nc.vector.tensor_scalar_mul(ot[:], ex[:], rc[:, 0:1])
                    nc.sync.dma_start(out=OUT[j * 128:(j + 1) * 128, :], in_=ot[:])
    nc.compile()
    return nc


# ---------------------------------------------------------------------------
# host-side preparation
# ---------------------------------------------------------------------------

def _perm_gates(w):
    """torch gate order (i,f,g,o) -> (f,i,o,g) along axis 0 (4H rows)."""
    Hq = w.shape[0] // 4
    i, f, g, o = (w[0:Hq], w[Hq:2 * Hq], w[2 * Hq:3 * Hq], w[3 * Hq:4 * Hq])
    return np.concatenate([f, i, o, g], axis=0)


def _bd_stream(wT_f, wT_b, bias_f, bias_b, kpad):
    """Block-diag stream [2*kpad, G4]: rows [0:K] = wT_f, [kpad-1] = bias_f, ..."""
    K = wT_f.shape[0]
    out = np.zeros((2 * kpad, wT_f.shape[1]), np.float32)
    out[0:K] = wT_f
    out[kpad - 1] = bias_f
    out[kpad:kpad + K] = wT_b
    out[2 * kpad - 1] = bias_b
    return out


def prepare_inputs(inputs, T, n_cores):
    bf = ml_dtypes.bfloat16
    C_ = np.asarray(inputs["confs"]).shape[1]
    emb = np.asarray(inputs["emb"], np.float32)
    tokens = np.asarray(inputs["tokens"])
    confs = np.asarray(inputs["confs"])

    p = {}
    p["emb"] = emb.astype(bf)

    Wih0f = _perm_gates(np.asarray(inputs["Wih0f"], np.float32))
    Wih0b = _perm_gates(np.asarray(inputs["Wih0b"], np.float32))
    b0f = _perm_gates(np.asarray(inputs["b0f"], np.float32))
    b0b = _perm_gates(np.asarray(inputs["b0b"], np.float32))
    Whh0f = _perm_gates(np.asarray(inputs["Whh0f"], np.float32))
    Whh0b = _perm_gates(np.asarray(inputs["Whh0b"], np.float32))
    Wih1f = _perm_gates(np.asarray(inputs["Wih1f"], np.float32))
    Wih1b = _perm_gates(np.asarray(inputs["Wih1b"], np.float32))
    b1f = _perm_gates(np.asarray(inputs["b1f"], np.float32))
    b1b = _perm_gates(np.asarray(inputs["b1b"], np.float32))
    Whh1f = _perm_gates(np.asarray(inputs["Whh1f"], np.float32))
    Whh1b = _perm_gates(np.asarray(inputs["Whh1b"], np.float32))
    w1 = np.asarray(inputs["w1"], np.float32)
    bw1 = np.asarray(inputs["bw1"], np.float32)
    w2 = np.asarray(inputs["w2"], np.float32)
    bw2 = np.asarray(inputs["bw2"], np.float32)

    p["W0s"] = _bd_stream(Wih0f.T, Wih0b.T, b0f, b0b, EP).astype(bf)
    p["Whh0s"] = _bd_stream(Whh0f.T, Whh0b.T, 0 * b0f, 0 * b0b, HP).astype(bf)
    p["W1sf"] = np.concatenate([Wih1f.T, b1f[None, :]], 0).astype(bf)
    p["W1sb"] = np.concatenate([Wih1b.T, b1b[None, :]], 0).astype(bf)
    p["Whh1s"] = _bd_stream(Whh1f.T, Whh1b.T, 0 * b1f, 0 * b1b, HP).astype(bf)
    wu = np.concatenate([np.concatenate([w1[:, 0:2 * H].T, w1[:, 2 * H:].T], 1),
                         np.zeros((1, 2 * G4 // 2), np.float32)], 0)
    p["WU"] = wu.astype(bf)
    p["bw1m"] = np.tile(bw1[None, :], (128, 1)).astype(np.float32)
    w2p = np.zeros((512, 4), np.float32)
    w2p[0:2 * H] = w2.T
    w2p[511] = bw2
    p["W2s"] = w2p.astype(bf)

    NCH = T // CHT
    NP = BL * C_
    NPT = (NP + 127) // 128

    in_maps = []
    for c in range(n_cores):
        m = dict(p)
        bs = tokens[c * BL:(c + 1) * BL, 0:T]          # [BL, T]
        tf = np.zeros((CHT * BL, NCH), np.int32)
        tb = np.zeros((CHT * BL, NCH), np.int32)
        for k in range(NCH):
            for tr in range(CHT):
                tf[tr * BL:(tr + 1) * BL, k] = bs[:, k * CHT + tr]
                tb[tr * BL:(tr + 1) * BL, k] = bs[:, T - 1 - (k * CHT + tr)]
        m["tokf"] = tf
        m["tokb"] = tb
        cf = confs[c * BL:(c + 1) * BL]                 # [BL, C, 2]
        t0 = cf[:, :, 0].reshape(-1)                    # row-major (b, ci)
        t1 = cf[:, :, 1].reshape(-1)
        bidx = np.repeat(np.arange(BL), C_)
        ui0 = np.clip(t0, 0, T - 1) * BL + bidx
        ui1 = np.clip(t1, 0, T - 1) * BL + bidx
        um0 = (t0 >= 0).astype(np.float32)
        um1 = (t1 >= 0).astype(np.float32)

        def tile128(a, dt):
            o = np.zeros((NPT * 128,), dt)
            o[:a.shape[0]] = a
            return o.reshape(NPT, 128).T.copy()
        m["uidx0"] = tile128(ui0.astype(np.int32), np.int32)
        m["uidx1"] = tile128(ui1.astype(np.int32), np.int32)
        m["umask0"] = tile128(um0, np.float32)
        m["umask1"] = tile128(um1, np.float32)
        in_maps.append(m)
    return in_maps


_CACHE = {}


def _get_prog(T, n_cores, NPT):
    key = (T, n_cores, NPT)
    if key not in _CACHE:
        _CACHE[key] = build(T, n_cores, NPT)
    return _CACHE[key]


def kernel(**inputs):
    T = inputs["tokens"].shape[1]
    C_ = inputs["confs"].shape[1]
    n_cores = NCORE
    NP = BL * C_
    NPT = (NP + 127) // 128
    nc = _get_prog(T, n_cores, NPT)
    in_maps = prepare_inputs(inputs, T, n_cores)
    res = run_bass_kernel_spmd(nc, in_maps, list(range(n_cores)))
    outs = []
    for c in range(n_cores):
        o = res.results[c]["OUT"][:NP]          # [BL*C, 4] rows (b, ci)
        outs.append(o)
    return np.concatenate(outs, axis=0).astype(np.float32)


# revision 12
# speedup vs baseline: 12.3880x; 12.3880x over previous
"""Trainium2 Bass kernel for nn_BiLSTMNet (2-layer BiLSTM + pair-gather MLP).

Strategy: fully data-parallel across 8 cores (16 sentences each), both LSTM
directions fused per core via block-diagonal matmuls.  Input projections are
computed just-in-time into PSUM (128-slot chunks); the recurrent matmul
accumulates on top (start=False), so gate pre-activations never touch DVE.
h^T is produced by DMA-transpose (bf16) and stored to DRAM in both processing
and reverse order so layer-1 / MLP consumers always read ascending columns.
MLP is decomposed as U0 = h1 @ w1[:, :2H].T, U1 = h1 @ w1[:, 2H:].T computed
for all (t, b), then the conf-pair gather is a row gather + add + tanh.
"""
import sys
sys.path.insert(0, "/opt/trn_rl_repo")
import numpy as np
import ml_dtypes

import concourse.bass as bass
import concourse.tile as tile
from concourse import mybir, bacc
from concourse.bass_utils import run_bass_kernel_spmd

BF16 = mybir.dt.bfloat16
F32 = mybir.dt.float32
I32 = mybir.dt.int32
AF = mybir.ActivationFunctionType
ALU = mybir.AluOpType

V, E, H, B, C = 32000, 200, 200, 128, 256
T_FULL = 512
BL = 16            # sentences per core
NCORE = 8
EP = 256           # padded E (dma-transpose wants 128-col blocks)
HP = 256           # padded H
G4 = 800           # 4*H gate width
CHT = 4            # timesteps per xg chunk (chunk = CHT*2*BL = 128 slots)


def build(T, n_cores, NPT):
    NCH = T // CHT
    NSLOT = T * BL                # per-direction (t,b) slots
    NUC = NSLOT // 128            # U-phase chunks

    nc = bacc.Bacc("TRN2", target_bir_lowering=False, debug=False,
                   enable_asserts=True, num_devices=n_cores)

    def din(name, shape, dt):
        return nc.dram_tensor(name, shape, dt, kind="ExternalInput").ap()

    def dout(name, shape, dt):
        return nc.dram_tensor(name, shape, dt, kind="ExternalOutput").ap()

    emb = din("emb", [V, E], BF16)
    W0s = din("W0s", [2 * EP, G4], BF16)      # xg0 stream (block-diag K rows)
    Whh0s = din("Whh0s", [2 * HP, G4], BF16)  # L0 recurrent stream
    W1sf = din("W1sf", [2 * H + 1, G4], BF16)  # xg1 stream, fwd block
    W1sb = din("W1sb", [2 * H + 1, G4], BF16)  # xg1 stream, bwd block
    Whh1s = din("Whh1s", [2 * HP, G4], BF16)
    WU = din("WU", [2 * H + 1, G4], BF16)      # U stream [w1a.T | w1b.T] + bw1 row
    W2s = din("W2s", [4 * 128, 4], BF16)       # w2.T padded to 512 rows + bw2 at 511
    tokf = din("tokf", [CHT * BL, NCH], I32)   # [slot, chunk]
    tokb = din("tokb", [CHT * BL, NCH], I32)
    uidx0 = din("uidx0", [128, NPT], I32)
    uidx1 = din("uidx1", [128, NPT], I32)
    umask0 = din("umask0", [128, NPT], F32)
    umask1 = din("umask1", [128, NPT], F32)
    bw1m = din("bw1m", [128, 2 * H], F32)

    OUT = dout("OUT", [NPT * 128, 4], F32)

    # internal DRAM
    # h0T rows: [0:200] f-proc | [200:400] b-rev | [400] ones |
    #           [401:601] f-rev | [601:801] b-proc | [801] ones
    h0T = nc.dram_tensor("h0T", [802, NSLOT], BF16).ap()
    # h1T rows: [0:200] f-proc | [200:400] b-rev | [400] ones
    h1T = nc.dram_tensor("h1T", [401, NSLOT], BF16).ap()
    U0 = nc.dram_tensor("U0", [NSLOT, 2 * H], F32).ap()
    U1 = nc.dram_tensor("U1", [NSLOT, 2 * H], F32).ap()

    with tile.TileContext(nc) as tc:
        with tc.tile_pool(name="const", bufs=1) as cp, \
             tc.tile_pool(name="state", bufs=1) as sp:

            # ---- load weight streams into SBUF K-chunk tiles
            def load_stream(src, nrows, ncols):
                tiles = []
                r = 0
                while r < nrows:
                    h_ = min(128, nrows - r)
                    t_ = cp.tile([h_, ncols], BF16, tag=f"st{src.name}{r}", name=f"st{src.name}{r}")
                    nc.sync.dma_start(out=t_[:], in_=src[r:r + h_, :])
                    tiles.append(t_)
                    r += h_
                return tiles

            W0t = load_stream(W0s.tensor.ap(), 2 * EP, G4)      # 4 x [128, 800]
            Whh0t = load_stream(Whh0s.tensor.ap(), 2 * HP, G4)  # 4
            W1ft = load_stream(W1sf.tensor.ap(), 2 * H + 1, G4)  # [128,128,128,17]
            W1bt = load_stream(W1sb.tensor.ap(), 2 * H + 1, G4)
            Whh1t = load_stream(Whh1s.tensor.ap(), 2 * HP, G4)
            WUt = load_stream(WU.tensor.ap(), 2 * H + 1, G4)
            W2t = load_stream(W2s.tensor.ap(), 4 * 128, 4)       # 4 x [128, 4]

            # token index tiles (slot-major: [64, NCH])
            tokf_t = cp.tile([CHT * BL, NCH], I32)
            tokb_t = cp.tile([CHT * BL, NCH], I32)
            nc.sync.dma_start(out=tokf_t[:], in_=tokf[:])
            nc.sync.dma_start(out=tokb_t[:], in_=tokb[:])

            # ones rows in h0T/h1T (bias rows consumed via lhsT chunk DMAs)
            ones_row = cp.tile([1, NSLOT], BF16)
            nc.vector.memset(ones_row[:], 1.0)
            nc.sync.dma_start(out=h0T[400:401, :], in_=ones_row[:])
            nc.sync.dma_start(out=h0T[801:802, :], in_=ones_row[:])
            nc.sync.dma_start(out=h1T[400:401, :], in_=ones_row[:])

            # ---- persistent state tiles
            # x gather tiles (per chunk parity): cols 200:255 zero, col 255 one
            xf = [sp.tile([CHT * BL, EP], BF16, tag=f"xf{i}", name=f"xf{i}") for i in range(2)]
            xb = [sp.tile([CHT * BL, EP], BF16, tag=f"xb{i}", name=f"xb{i}") for i in range(2)]
            for t_ in xf + xb:
                nc.vector.memset(t_[:], 0.0)
                nc.vector.memset(t_[:, EP - 1:EP], 1.0)
            # xg lhsT tiles (block-diag): C0..C3 per parity
            Ct = [[sp.tile([128, 128], BF16, tag=f"C{i}{j}", name=f"C{i}{j}") for i in range(4)]
                  for j in range(2)]
            # rec lhsT tiles A0..A3 per step parity
            At = [[sp.tile([128, 2 * BL], BF16, tag=f"A{i}{j}", name=f"A{i}{j}") for i in range(4)]
                  for j in range(2)]
            # L1 xg lhsT tiles D0..D7 per parity (last of each block is 17 rows)
            Dt = [[sp.tile([17 if i in (3, 7) else 128, 128], BF16, tag=f"D{i}{j}", name=f"D{i}{j}")
                   for i in range(8)] for j in range(2)]
            for j in range(2):
                for t_ in Ct[j] + At[j] + Dt[j]:
                    nc.vector.memset(t_[:], 0.0)
            # LSTM state: S = [c | tg] fp32; h per parity
            S = sp.tile([2 * BL, 2 * H], F32)
            ht = [sp.tile([2 * BL, HP], BF16, tag=f"h{i}", name=f"h{i}") for i in range(2)]
            for t_ in ht:
                nc.vector.memset(t_[:], 0.0)
            # identity for PE transposes
            ident32 = sp.tile([32, 32], BF16, name="ident32")
            from concourse.masks import make_identity
            make_identity(nc, ident32[:])

            NB = 2 * BL  # 32 rows per step (f+b)

            with tc.tile_pool(name="work", bufs=2) as wp, \
                 tc.tile_pool(name="xps", bufs=2, space="PSUM") as xps, \
                 tc.tile_pool(name="tps", bufs=2, space="PSUM") as tps:

                xg_tiles = {}

                def emit_xg0_chunk(k):
                    par = k % 2
                    gxf = xf[par]
                    gxb = xb[par]
                    nc.gpsimd.indirect_dma_start(
                        out=gxf[:, 0:E], out_offset=None, in_=emb[:],
                        in_offset=bass.IndirectOffsetOnAxis(ap=tokf_t[:, k:k + 1], axis=0))
                    nc.gpsimd.indirect_dma_start(
                        out=gxb[:, 0:E], out_offset=None, in_=emb[:],
                        in_offset=bass.IndirectOffsetOnAxis(ap=tokb_t[:, k:k + 1], axis=0))
                    # transpose x -> staging, then strided copy into C tiles
                    for i, (src, coff) in enumerate([(gxf, 0), (gxb, BL)]):
                        for half in range(2):
                            stg = wp.tile([128, CHT * BL], BF16, tag="xstg", name="xstg")
                            nc.sync.dma_start_transpose(
                                out=stg[:], in_=src[:, half * 128:half * 128 + 128])
                            ctile = Ct[par][2 * i + half]
                            dst = ctile[:].rearrange("p (a b) -> p a b", b=NB)[:, :, coff:coff + BL]
                            s3 = stg[:].rearrange("p (a b) -> p a b", b=BL)
                            nc.vector.tensor_copy(dst, s3)
                    xg = xps.tile([128, G4], F32, space="PSUM", tag="xg", name="xg")
                    xg_tiles[("L0", k)] = xg
                    for kc in range(4):
                        for (n0, n1) in ((0, 512), (512, G4)):
                            nc.tensor.matmul(xg[:, n0:n1], Ct[par][kc][:],
                                             W0t[kc][:, n0:n1],
                                             start=(kc == 0), stop=(kc == 3))

                def emit_xg1_chunk(k):
                    par = k % 2
                    c0 = k * CHT * BL
                    cw = CHT * BL
                    rowsets = [(0, 128), (128, 256), (256, 384), (384, 401),
                               (401, 529), (529, 657), (657, 785), (785, 802)]
                    for i, (r0, r1) in enumerate(rowsets):
                        dtile = Dt[par][i]
                        coff = 0 if i < 4 else BL
                        dst = dtile[:].rearrange("p (a b) -> p a b", b=NB)[:, :, coff:coff + BL]
                        src = h0T[r0:r1, c0:c0 + cw].rearrange("p (a b) -> p a b", b=BL)
                        nc.sync.dma_start(out=dst, in_=src)
                    xg = xps.tile([128, G4], F32, space="PSUM", tag="xg", name="xg")
                    xg_tiles[("L1", k)] = xg
                    streams = [W1ft[0], W1ft[1], W1ft[2], W1ft[3],
                               W1bt[0], W1bt[1], W1bt[2], W1bt[3]]
                    for kc in range(8):
                        for (n0, n1) in ((0, 512), (512, G4)):
                            nc.tensor.matmul(xg[:, n0:n1], Dt[par][kc][:],
                                             streams[kc][:, n0:n1],
                                             start=(kc == 0), stop=(kc == 7))

                def emit_step(layer, p, T_, Whht, store_all):
                    par = p % 2
                    k = p // CHT
                    r = (p % CHT) * NB
                    xg = xg_tiles[(layer, k)]
                    # recurrent matmul accumulating onto the xg psum slice
                    # (N-outer so the sigmoid can start after the first N-group)
                    for (n0, n1) in ((0, 512), (512, G4)):
                        for kc in range(4):
                            nc.tensor.matmul(xg[r:r + NB, n0:n1],
                                             At[(p + 1) % 2][kc][:],
                                             Whht[kc][:, n0:n1],
                                             start=False, stop=(kc == 3),
                                             skip_group_check=True,
                                             tile_position=(0, r))
                    # gate nonlinearities (gate order f,i,o,g)
                    sigs = wp.tile([NB, 600], F32, tag="sigs", name="sigs")
                    nc.scalar.activation(sigs[:, 0:2 * H], xg[r:r + NB, 0:2 * H],
                                         AF.Sigmoid)
                    nc.scalar.activation(S[:, H:2 * H], xg[r:r + NB, 600:800], AF.Tanh)
                    prod = wp.tile([NB, 2 * H], F32, tag="prod", name="prod")
                    nc.vector.tensor_mul(prod[:], sigs[:, 0:2 * H], S[:, 0:2 * H])
                    nc.vector.tensor_add(S[:, 0:H], prod[:, 0:H], prod[:, H:2 * H])
                    # sigma(o) off the c-critical path
                    nc.scalar.activation(sigs[:, 2 * H:600], xg[r:r + NB, 2 * H:600],
                                         AF.Sigmoid)
                    tct = wp.tile([NB, H], F32, tag="tct", name="tct")
                    nc.scalar.activation(tct[:], S[:, 0:H], AF.Tanh)
                    hcur = ht[par]
                    nc.vector.tensor_mul(hcur[:, 0:H], sigs[:, 400:600], tct[:])
                    # transpose h via PE -> PSUM, copy slices to next-step lhsT
                    ps1 = tps.tile([128, NB], BF16, space="PSUM", tag="ps1", name="ps1")
                    ps2 = tps.tile([72, NB], BF16, space="PSUM", tag="ps2", name="ps2")
                    nc.tensor.transpose(ps1[:], hcur[:, 0:128], ident32[:])
                    nc.tensor.transpose(ps2[:], hcur[:, 128:200], ident32[:])
                    nA = At[par]
                    nc.vector.tensor_copy(nA[0][:, 0:BL], ps1[:, 0:BL])
                    nc.scalar.copy(nA[1][0:72, 0:BL], ps2[:, 0:BL])
                    nc.vector.tensor_copy(nA[2][:, BL:NB], ps1[:, BL:NB])
                    nc.scalar.copy(nA[3][0:72, BL:NB], ps2[:, BL:NB])
                    # h^T stores from the A tiles (SBUF), off the critical path
                    hT = h0T if layer == "L0" else h1T
                    cp_ = p * BL
                    cr = (T_ - 1 - p) * BL
                    # f-proc rows 0:200 at processing col
                    nc.sync.dma_start(out=hT[0:128, cp_:cp_ + BL], in_=nA[0][:, 0:BL])
                    nc.sync.dma_start(out=hT[128:200, cp_:cp_ + BL], in_=nA[1][0:72, 0:BL])
                    # b-rev rows 200:400 at reversed col
                    nc.sync.dma_start(out=hT[200:328, cr:cr + BL], in_=nA[2][:, BL:NB])
                    nc.sync.dma_start(out=hT[328:400, cr:cr + BL], in_=nA[3][0:72, BL:NB])
                    if store_all:
                        # f-rev rows 401:601, b-proc rows 601:801
                        nc.sync.dma_start(out=hT[401:529, cr:cr + BL], in_=nA[0][:, 0:BL])
                        nc.sync.dma_start(out=hT[529:601, cr:cr + BL], in_=nA[1][0:72, 0:BL])
                        nc.sync.dma_start(out=hT[601:729, cp_:cp_ + BL], in_=nA[2][:, BL:NB])
                        nc.sync.dma_start(out=hT[729:801, cp_:cp_ + BL], in_=nA[3][0:72, BL:NB])

                def reset_states():
                    nc.vector.memset(S[:], 0.0)
                    for j in range(2):
                        for t_ in At[j]:
                            nc.vector.memset(t_[:], 0.0)

                # ================= layer 0 =================
                reset_states()
                emit_xg0_chunk(0)
                for k in range(NCH):
                    if k + 1 < NCH:
                        emit_xg0_chunk(k + 1)
                    for tr in range(CHT):
                        emit_step("L0", k * CHT + tr, T, Whh0t, True)

                # ================= layer 1 =================
                reset_states()
                emit_xg1_chunk(0)
                for k in range(NCH):
                    if k + 1 < NCH:
                        emit_xg1_chunk(k + 1)
                    for tr in range(CHT):
                        emit_step("L1", k * CHT + tr, T, Whh1t, False)

            # ================= U phase =================
            with tc.tile_pool(name="uw", bufs=2) as uw, \
                 tc.tile_pool(name="ups", bufs=2, space="PSUM") as ups:
                rowsets = [(0, 128), (128, 256), (256, 384), (384, 401)]
                for k in range(NUC):
                    c0 = k * 128
                    et = []
                    for (r0, r1) in rowsets:
                        t_ = uw.tile([r1 - r0, 128], BF16, tag=f"E{r0}", name=f"E{r0}")
                        nc.sync.dma_start(out=t_[:], in_=h1T[r0:r1, c0:c0 + 128])
                        et.append(t_)
                    psu = ups.tile([128, G4], F32, space="PSUM", tag="psu", name="psu")
                    for kc in range(4):
                        for (n0, n1) in ((0, 512), (512, G4)):
                            nc.tensor.matmul(psu[:, n0:n1], et[kc][:],
                                             WUt[kc][:, n0:n1],
                                             start=(kc == 0), stop=(kc == 3))
                    uo = uw.tile([128, G4], F32, tag="uo", name="uo")
                    nc.vector.tensor_copy(uo[:], psu[:])
                    nc.sync.dma_start(out=U0[c0:c0 + 128, :], in_=uo[:, 0:2 * H])
                    nc.sync.dma_start(out=U1[c0:c0 + 128, :], in_=uo[:, 2 * H:G4])

            # ================= final gather + MLP =================
            with tc.tile_pool(name="fw", bufs=2) as fw, \
                 tc.tile_pool(name="fc", bufs=1) as fc, \
                 tc.tile_pool(name="fps", bufs=2, space="PSUM") as fps:
                ui0 = fc.tile([128, NPT], I32)
                ui1 = fc.tile([128, NPT], I32)
                um0 = fc.tile([128, NPT], F32)
                um1 = fc.tile([128, NPT], F32)
                nc.sync.dma_start(out=ui0[:], in_=uidx0[:])
                nc.sync.dma_start(out=ui1[:], in_=uidx1[:])
                nc.sync.dma_start(out=um0[:], in_=umask0[:])
                nc.sync.dma_start(out=um1[:], in_=umask1[:])
                bwt = fc.tile([128, 2 * H], F32, name="bwt")
                nc.sync.dma_start(out=bwt[:], in_=bw1m[:])
                hm = [fc.tile([128, 512], BF16, tag=f"hm{i}", name=f"hm{i}") for i in range(2)]
                for t_ in hm:
                    nc.vector.memset(t_[:], 0.0)
                    nc.vector.memset(t_[:, 511:512], 1.0)
                for j in range(NPT):
                    par = j % 2
                    g0 = fw.tile([128, 2 * H], F32, tag="g0", name="g0")
                    g1 = fw.tile([128, 2 * H], F32, tag="g1", name="g1")
                    nc.gpsimd.indirect_dma_start(
                        out=g0[:], out_offset=None, in_=U0[:],
                        in_offset=bass.IndirectOffsetOnAxis(ap=ui0[:, j:j + 1], axis=0))
                    nc.gpsimd.indirect_dma_start(
                        out=g1[:], out_offset=None, in_=U1[:],
                        in_offset=bass.IndirectOffsetOnAxis(ap=ui1[:, j:j + 1], axis=0))
                    g1m = fw.tile([128, 2 * H], F32, tag="g1m", name="g1m")
                    nc.vector.scalar_tensor_tensor(g1m[:], g1[:], um1[:, j:j + 1],
                                                   bwt[:], ALU.mult, ALU.add)
                    ssum = fw.tile([128, 2 * H], F32, tag="ssum", name="ssum")
                    nc.vector.scalar_tensor_tensor(ssum[:], g0[:], um0[:, j:j + 1],
                                                   g1m[:], ALU.mult, ALU.add)
                    nc.scalar.activation(hm[par][:, 0:2 * H], ssum[:], AF.Tanh)
                    hmT = []
                    for i in range(4):
                        t_ = fw.tile([128, 128], BF16, tag=f"hmT{i}", name=f"hmT{i}")
                        nc.sync.dma_start_transpose(
                            out=t_[:], in_=hm[par][:, i * 128:(i + 1) * 128])
                        hmT.append(t_)
                    psl = fps.tile([128, 4], F32, space="PSUM", tag="psl", name="psl")
                    for i in range(4):
                        nc.tensor.matmul(psl[:], hmT[i][:], W2t[i][:],
                                         start=(i == 0), stop=(i == 3))
                    ex = fw.tile([128, 4], F32, tag="ex", name="ex")
                    nc.scalar.activation(ex[:], psl[:], AF.Exp)
                    sm = fw.tile([128, 1], F32, tag="sm", name="sm")
                    nc.vector.reduce_sum(sm[:], ex[:], axis=mybir.AxisListType.X)
                    rc = fw.tile([128, 1], F32, tag="rc", name="rc")
                    nc.vector.reciprocal(rc[:], sm[:])
                    ot = fw.tile([128, 4], F32, tag="ot", name="ot")
                    nc.vector.tensor_scalar_mul(ot[:], ex[:], rc[:, 0:1])
                    nc.sync.dma_start(out=OUT[j * 128:(j + 1) * 128, :], in_=ot[:])
    nc.compile()
    return nc


# ---------------------------------------------------------------------------
# host-side preparation
# ---------------------------------------------------------------------------

def _perm_gates(w):
    """torch gate order (i,f,g,o) -> (f,i,o,g) along axis 0 (4H rows)."""
    Hq = w.shape[0] // 4
    i, f, g, o = (w[0:Hq], w[Hq:2 * Hq], w[2 * Hq:3 * Hq], w[3 * Hq:4 * Hq])
    return np.concatenate([f, i, o, g], axis=0)


def _bd_stream(wT_f, wT_b, bias_f, bias_b, kpad):
    """Block-diag stream [2*kpad, G4]: rows [0:K] = wT_f, [kpad-1] = bias_f, ..."""
    K = wT_f.shape[0]
    out = np.zeros((2 * kpad, wT_f.shape[1]), np.float32)
    out[0:K] = wT_f
    out[kpad - 1] = bias_f
    out[kpad:kpad + K] = wT_b
    out[2 * kpad - 1] = bias_b
    return out


def prepare_inputs(inputs, T, n_cores):
    bf = ml_dtypes.bfloat16
    C_ = np.asarray(inputs["confs"]).shape[1]
    emb = np.asarray(inputs["emb"], np.float32)
    tokens = np.asarray(inputs["tokens"])
    confs = np.asarray(inputs["confs"])

    p = {}
    p["emb"] = emb.astype(bf)

    Wih0f = _perm_gates(np.asarray(inputs["Wih0f"], np.float32))
    Wih0b = _perm_gates(np.asarray(inputs["Wih0b"], np.float32))
    b0f = _perm_gates(np.asarray(inputs["b0f"], np.float32))
    b0b = _perm_gates(np.asarray(inputs["b0b"], np.float32))
    Whh0f = _perm_gates(np.asarray(inputs["Whh0f"], np.float32))
    Whh0b = _perm_gates(np.asarray(inputs["Whh0b"], np.float32))
    Wih1f = _perm_gates(np.asarray(inputs["Wih1f"], np.float32))
    Wih1b = _perm_gates(np.asarray(inputs["Wih1b"], np.float32))
    b1f = _perm_gates(np.asarray(inputs["b1f"], np.float32))
    b1b = _perm_gates(np.asarray(inputs["b1b"], np.float32))
    Whh1f = _perm_gates(np.asarray(inputs["Whh1f"], np.float32))
    Whh1b = _perm_gates(np.asarray(inputs["Whh1b"], np.float32))
    w1 = np.asarray(inputs["w1"], np.float32)
    bw1 = np.asarray(inputs["bw1"], np.float32)
    w2 = np.asarray(inputs["w2"], np.float32)
    bw2 = np.asarray(inputs["bw2"], np.float32)

    p["W0s"] = _bd_stream(Wih0f.T, Wih0b.T, b0f, b0b, EP).astype(bf)
    p["Whh0s"] = _bd_stream(Whh0f.T, Whh0b.T, 0 * b0f, 0 * b0b, HP).astype(bf)
    p["W1sf"] = np.concatenate([Wih1f.T, b1f[None, :]], 0).astype(bf)
    p["W1sb"] = np.concatenate([Wih1b.T, b1b[None, :]], 0).astype(bf)
    p["Whh1s"] = _bd_stream(Whh1f.T, Whh1b.T, 0 * b1f, 0 * b1b, HP).astype(bf)
    wu = np.concatenate([np.concatenate([w1[:, 0:2 * H].T, w1[:, 2 * H:].T], 1),
                         np.zeros((1, 2 * G4 // 2), np.float32)], 0)
    p["WU"] = wu.astype(bf)
    p["bw1m"] = np.tile(bw1[None, :], (128, 1)).astype(np.float32)
    w2p = np.zeros((512, 4), np.float32)
    w2p[0:2 * H] = w2.T
    w2p[511] = bw2
    p["W2s"] = w2p.astype(bf)

    NCH = T // CHT
    NP = BL * C_
    NPT = (NP + 127) // 128

    in_maps = []
    for c in range(n_cores):
        m = dict(p)
        bs = tokens[c * BL:(c + 1) * BL, 0:T]          # [BL, T]
        tf = np.zeros((CHT * BL, NCH), np.int32)
        tb = np.zeros((CHT * BL, NCH), np.int32)
        for k in range(NCH):
            for tr in range(CHT):
                tf[tr * BL:(tr + 1) * BL, k] = bs[:, k * CHT + tr]
                tb[tr * BL:(tr + 1) * BL, k] = bs[:, T - 1 - (k * CHT + tr)]
        m["tokf"] = tf
        m["tokb"] = tb
        cf = confs[c * BL:(c + 1) * BL]                 # [BL, C, 2]
        t0 = cf[:, :, 0].reshape(-1)                    # row-major (b, ci)
        t1 = cf[:, :, 1].reshape(-1)
        bidx = np.repeat(np.arange(BL), C_)
        ui0 = np.clip(t0, 0, T - 1) * BL + bidx
        ui1 = np.clip(t1, 0, T - 1) * BL + bidx
        um0 = (t0 >= 0).astype(np.float32)
        um1 = (t1 >= 0).astype(np.float32)

        def tile128(a, dt):
            o = np.zeros((NPT * 128,), dt)
            o[:a.shape[0]] = a
            return o.reshape(NPT, 128).T.copy()
        m["uidx0"] = tile128(ui0.astype(np.int32), np.int32)
        m["uidx1"] = tile128(ui1.astype(np.int32), np.int32)
        m["umask0"] = tile128(um0, np.float32)
        m["umask1"] = tile128(um1, np.float32)
        in_maps.append(m)
    return in_maps


_CACHE = {}


def _get_prog(T, n_cores, NPT):
    key = (T, n_cores, NPT)
    if key not in _CACHE:
        _CACHE[key] = build(T, n_cores, NPT)
    return _CACHE[key]


def kernel(**inputs):
    T = inputs["tokens"].shape[1]
    C_ = inputs["confs"].shape[1]
    n_cores = NCORE
    NP = BL * C_
    NPT = (NP + 127) // 128
    nc = _get_prog(T, n_cores, NPT)
    in_maps = prepare_inputs(inputs, T, n_cores)
    res = run_bass_kernel_spmd(nc, in_maps, list(range(n_cores)))
    outs = []
    for c in range(n_cores):
        o = res.results[c]["OUT"][:NP]          # [BL*C, 4] rows (b, ci)
        outs.append(o)
    return np.concatenate(outs, axis=0).astype(np.float32)
